# revision 1
# baseline (speedup 1.0000x reference)
"""Trainium2 Bass kernel for nn_Loss_2482491097912 (SimCLR-style semi-supervised loss).

Strategy (8 NeuronCores, data-parallel over anchor rows):
  - Each core receives the FULL z1/z2 (pre-transposed + bf16 on host) and its
    1024-row slice of the masks, with h-columns ROLLED so that every core's
    local rows sit at columns [0:1024] -> one shared SPMD program.
  - On-core: projection MLP in transposed layout (HT = [feat, rows]),
    column-normalize (1/||h|| via Ln/Exp), then 4 sim matmuls
    (S11/S12/S21/S22 rows of this core's block vs all columns), exp with
    fused row-sum accumulation on the Scalar engine, fused masked sums
    (tensor_tensor_reduce) on the Vector engine.
  - Host combines per-core row sums / masked sums / diag dots into the two
    scalar losses (cheap numpy tail math).
"""

import numpy as np
import ml_dtypes

import concourse.bass as bass
import concourse.bacc as bacc
import concourse.tile as tile
import concourse.mybir as mybir
from concourse.bass_utils import run_bass_kernel_spmd

F32 = mybir.dt.float32
BF16 = mybir.dt.bfloat16
U8 = mybir.dt.uint8
AF = mybir.ActivationFunctionType
OP = mybir.AluOpType

N = 8192
D = 512
NCORES = 8
RPC = N // NCORES          # rows per core = 1024
NBLK = RPC // 128          # row blocks per core = 8
NH = 2 * N                 # 16384 stacked rows (h1 then h2)
CCH = 2048                 # sim column chunk
NCCH = N // CCH            # 4 chunks per sim row
PCH = 512                  # projection column chunk
NPCH = NH // PCH           # 32 projection chunks

# acc_all column layout: [4 sims x 8 blocks x 4 chunks] rowsums, then
# [4 mask-kinds x 8 blocks x 4 chunks] masked sums.
ACC_COLS = 256


def _emit(nc, tc, reps=1):
    zt = nc.dram_tensor("zt", [D, NH], BF16, kind="ExternalInput").ap()
    w1t = nc.dram_tensor("w1t", [D + 1, D], BF16, kind="ExternalInput").ap()
    w2t = nc.dram_tensor("w2t", [D, D], BF16, kind="ExternalInput").ap()
    b2t = nc.dram_tensor("b2t", [D, 1], F32, kind="ExternalInput").ap()
    pm = nc.dram_tensor("pm", [RPC, N], U8, kind="ExternalInput").ap()
    nm = nc.dram_tensor("nm", [RPC, N], U8, kind="ExternalInput").ap()

    acc_out = nc.dram_tensor("acc", [128, ACC_COLS], F32, kind="ExternalOutput").ap()
    d12_out = nc.dram_tensor("d12p", [1, RPC], F32, kind="ExternalOutput").ap()
    for _ in range(reps):
        _emit_body(nc, tc, zt, w1t, w2t, b2t, pm, nm, acc_out, d12_out)


def _emit_body(nc, tc, zt, w1t, w2t, b2t, pm, nm, acc_out, d12_out):

    zt_r = zt.rearrange("(t p) n -> p t n", p=128)      # [128, 4, 16384]
    w1t_r = w1t[0:D, :].rearrange("(t p) m -> p t m", p=128)
    w2t_r = w2t.rearrange("(t p) m -> p t m", p=128)
    b2t_r = b2t.rearrange("(t p) 1 -> p t", p=128)      # [128, 4]

    top = tc.alloc_tile_pool(name="top", bufs=1)
    # resident normalized H^T, 4 k-tiles of [128, 16384] bf16 (32KB/part each)
    ht = [top.tile([128, NH], BF16, name=f"ht{k}", tag=f"ht{k}") for k in range(4)]
    acc_all = top.tile([128, ACC_COLS], F32, name="acc_all")
    w1_sb = top.tile([128, 4, D], BF16, name="w1_sb")
    b1_sb = top.tile([1, D], BF16, name="b1_sb")
    w2_sb = top.tile([128, 4, D], BF16, name="w2_sb")
    b2_sb = top.tile([128, 4], F32, name="b2_sb")
    ones_r = top.tile([1, D], BF16, name="ones_r")      # rhs for L1 bias matmul
    ones_c = top.tile([128, 1], F32, name="ones_c")     # f32 lhsT for d12 sums
    ones_cb = top.tile([128, 1], BF16, name="ones_cb")  # bf16 lhsT for norm sums

    nc.sync.dma_start(w1_sb[:], w1t_r)
    nc.sync.dma_start(b1_sb[:], w1t[D:D + 1, :])
    nc.sync.dma_start(w2_sb[:], w2t_r)
    nc.sync.dma_start(b2_sb[:], b2t_r)
    nc.vector.memset(ones_r[:], 1.0)
    nc.vector.memset(ones_c[:], 1.0)
    nc.vector.memset(ones_cb[:], 1.0)
    nc.vector.memset(acc_all[:], 0.0)

    # ---------------- Phase 1: projection (transposed layout) ----------------
    with (
        tc.tile_pool(name="pj_sb", bufs=2) as pj,
        tc.tile_pool(name="pp_l1", bufs=2, space="PSUM") as pp_l1,
        tc.tile_pool(name="pp_l2", bufs=1, space="PSUM") as pp_l2,
        tc.tile_pool(name="pp_n", bufs=2, space="PSUM") as pp_n,
    ):
        for c in range(NPCH):
            cs = c * PCH
            zt_t = pj.tile([128, 4, PCH], BF16, name="zt_t", tag="zt")
            nc.sync.dma_start(zt_t[:], zt_r[:, :, cs:cs + PCH])

            gts = []
            for m in range(4):
                ms = m * 128
                l1_ps = pp_l1.tile([128, PCH], F32, name="l1_ps", tag="l1")
                for k in range(4):
                    nc.tensor.matmul(
                        l1_ps[:], w1_sb[:, k, ms:ms + 128], zt_t[:, k, :],
                        start=(k == 0), stop=False)
                # bias row via K=1 matmul: adds b1[m-chunk] to all columns
                nc.tensor.matmul(
                    l1_ps[:], b1_sb[:, ms:ms + 128], ones_r[:, 0:PCH],
                    start=False, stop=True)
                t_sb = pj.tile([128, PCH], F32, name="t_sb", tag="texp")
                nc.scalar.activation(t_sb[:], l1_ps[:], AF.Exp)
                gt = pj.tile([128, PCH], BF16, name="gt", tag=f"gt{m}")
                # elu(u) = min(exp(u) - 1, u)
                nc.vector.scalar_tensor_tensor(
                    gt[:], t_sb[:], 1.0, l1_ps[:], op0=OP.subtract, op1=OP.min)
                gts.append(gt)

            norms_ps = pp_n.tile([1, PCH], F32, name="norms_ps", tag="n")
            l2s = []
            for m in range(4):
                ms = m * 128
                l2_ps = pp_l2.tile([128, PCH], F32, name="l2_ps", tag=f"l2{m}")
                for k in range(4):
                    nc.tensor.matmul(
                        l2_ps[:], w2_sb[:, k, ms:ms + 128], gts[k][:],
                        start=(k == 0), stop=(k == 3))
                sq_sb = pj.tile([128, PCH], BF16, name="sq_sb", tag="sq")
                nc.scalar.activation(
                    sq_sb[:], l2_ps[:], AF.Square, bias=b2_sb[:, m:m + 1])
                nc.tensor.matmul(
                    norms_ps[:], ones_cb[:], sq_sb[:],
                    start=(m == 0), stop=(m == 3))
                l2s.append(l2_ps)

            # r = 1/sqrt(norms^2) = exp(-0.5 * ln(norms^2)), then broadcast
            lg = pj.tile([1, PCH], F32, name="lg", tag="lg")
            nc.scalar.activation(lg[:], norms_ps[:], AF.Ln)
            r1 = pj.tile([1, PCH], F32, name="r1", tag="r1")
            nc.scalar.activation(r1[:], lg[:], AF.Exp, scale=-0.5)
            rb = pj.tile([128, PCH], F32, name="rb", tag="rb")
            nc.gpsimd.partition_broadcast(rb[:], r1[:])

            for m in range(4):
                # htn = (h + b2) * r  -> bf16 into resident HT
                nc.vector.scalar_tensor_tensor(
                    ht[m][:, cs:cs + PCH], l2s[m][:], b2_sb[:, m:m + 1], rb[:],
                    op0=OP.add, op1=OP.mult)

    # ---------------- Phase 1.5: d12 = rowwise dot n1.n2 for local rows ------
    with (
        tc.tile_pool(name="dd_sb", bufs=2) as dd,
        tc.tile_pool(name="dd_ps", bufs=2, space="PSUM") as dd_ps,
    ):
        d12_sb = dd.tile([1, RPC], F32, name="d12_sb", bufs=1)
        for h in range(2):
            hs = h * 512
            dps = dd_ps.tile([1, 512], F32, name="dps", tag="dps")
            for k in range(4):
                mt = dd.tile([128, 512], F32, name="mt", tag="mt")
                nc.vector.tensor_mul(
                    mt[:], ht[k][:, hs:hs + 512], ht[k][:, N + hs:N + hs + 512])
                nc.tensor.matmul(dps[:], ones_c[:], mt[:],
                                 start=(k == 0), stop=(k == 3))
            nc.scalar.copy(d12_sb[:, hs:hs + 512], dps[:])
        nc.sync.dma_start(d12_out[:], d12_sb[:])

    # ---------------- Phase 2: sims + exp row-sums + masked sums -------------
    # sim order: (mat_idx, lhs_half, rhs_half, masked)
    sims = [
        (1, 0, 1, True),    # S12
        (2, 1, 0, True),    # S21
        (0, 0, 0, False),   # S11
        (3, 1, 1, False),   # S22
    ]
    with (
        tc.tile_pool(name="sm_sb", bufs=2) as sm,
        tc.tile_pool(name="mk_sb", bufs=2) as mk,
        tc.tile_pool(name="sm_ps", bufs=2, space="PSUM") as sm_ps,
    ):
        for b in range(NBLK):
            pm_t = mk.tile([128, N], U8, name="pm_t", tag="pm")
            nc.sync.dma_start(pm_t[:], pm[b * 128:(b + 1) * 128, :])
            nm_t = mk.tile([128, N], U8, name="nm_t", tag="nm")
            nc.sync.dma_start(nm_t[:], nm[b * 128:(b + 1) * 128, :])
            for mat, lh, rh, masked in sims:
                lc = lh * N + b * 128
                for c in range(NCCH):
                    rcs = rh * N + c * CCH
                    s_ps = sm_ps.tile([128, CCH], F32, name="s_ps", tag="s")
                    for k in range(4):
                        for n in range(4):
                            ns = n * 512
                            nc.tensor.matmul(
                                s_ps[:, ns:ns + 512],
                                ht[k][:, lc:lc + 128],
                                ht[k][:, rcs + ns:rcs + ns + 512],
                                start=(k == 0), stop=(k == 3))
                    e_sb = sm.tile([128, CCH], F32, name="e_sb", tag="e")
                    col = mat * 32 + b * 4 + c
                    nc.scalar.activation(
                        e_sb[:], s_ps[:], AF.Exp, scale=2.0,
                        accum_out=acc_all[:, col:col + 1])
                    if masked:
                        mk_idx = 0 if mat == 1 else 2
                        for mki, m_t in ((mk_idx, pm_t), (mk_idx + 1, nm_t)):
                            mcol = 128 + mki * 32 + b * 4 + c
                            tsc = sm.tile([128, CCH], BF16, name="tsc",
                                          tag="tsc", bufs=1)
                            nc.vector.scalar_tensor_tensor(
                                tsc[:], e_sb[:], 1.0,
                                m_t[:, c * CCH:(c + 1) * CCH],
                                op0=OP.mult, op1=OP.mult,
                                accum_out=acc_all[:, mcol:mcol + 1])

        nc.sync.dma_start(acc_out[:], acc_all[:])
    top.release()


_CACHE = {}


def _build(reps=1):
    key = ("nc", reps)
    if key in _CACHE:
        return _CACHE[key]
    nc = bacc.Bacc("TRN2", target_bir_lowering=False, debug=False,
                   enable_asserts=False, num_devices=NCORES)
    with tile.TileContext(nc) as tc:
        _emit(nc, tc, reps=reps)
    nc.compile()
    _CACHE[key] = nc
    return nc


def prepare_in_maps(z1, z2, pos_mask, neg_mask, W1, b1, W2, b2):
    bf16 = ml_dtypes.bfloat16
    w1t_aug = np.concatenate([W1.T, b1[None, :]], axis=0).astype(bf16)
    w2t = np.ascontiguousarray(W2.T).astype(bf16)
    b2t = np.ascontiguousarray(b2[:, None]).astype(np.float32)
    pm_u8 = np.asarray(pos_mask).astype(np.uint8)
    nm_u8 = np.asarray(neg_mask).astype(np.uint8)

    in_maps = []
    for d in range(NCORES):
        r0 = d * RPC
        z1r = np.roll(z1, -r0, axis=0)
        z2r = np.roll(z2, -r0, axis=0)
        zt = np.ascontiguousarray(
            np.concatenate([z1r, z2r], axis=0).T).astype(bf16)
        pm_d = np.ascontiguousarray(np.roll(pm_u8[r0:r0 + RPC], -r0, axis=1))
        nm_d = np.ascontiguousarray(np.roll(nm_u8[r0:r0 + RPC], -r0, axis=1))
        in_maps.append({"zt": zt, "w1t": w1t_aug, "w2t": w2t, "b2t": b2t,
                        "pm": pm_d, "nm": nm_d})
    return in_maps


def finalize(results):
    """Host tail math (f64): per-core acc/d12p -> (unsup, semi)."""
    e2 = np.exp(2.0)
    rs = np.zeros((4, N))          # row sums of E11,E12,E21,E22 (global rows)
    mk = np.zeros(4)               # masked sums: p12, n12, p21, n21
    d12 = np.zeros(N)
    for d in range(NCORES):
        acc = results[d]["acc"].astype(np.float64)
        r0 = d * RPC
        for mat in range(4):
            cols = acc[:, mat * 32:(mat + 1) * 32].reshape(128, 8, 4).sum(2)
            rs[mat, r0:r0 + RPC] = cols.T.reshape(RPC)
        for mki in range(4):
            mk[mki] += acc[:, 128 + mki * 32:128 + (mki + 1) * 32].sum()
        d12[r0:r0 + RPC] = results[d]["d12p"][0].astype(np.float64)

    num = np.exp(2.0 * d12)
    rs11, rs12, rs21, rs22 = rs
    l1 = -np.log(num / (rs11 + rs12 - e2))
    l2 = -np.log(num / (rs22 + rs21 - e2))
    unsup = 0.5 * (l1 + l2).sum() / N

    tr = num.sum()
    p12, n12, p21, n21 = mk
    s1 = -np.log(p12 / (p12 + (n12 - tr)))
    s2 = -np.log(p21 / (p21 + (n21 - tr)))
    semi = 0.5 * (s1 + s2)

    return (np.float32(unsup), np.float32(semi))


def kernel(z1, z2, pos_mask, neg_mask, W1, b1, W2, b2):
    nc = _build()
    in_maps = prepare_in_maps(z1, z2, pos_mask, neg_mask, W1, b1, W2, b2)
    res = run_bass_kernel_spmd(nc, in_maps, core_ids=list(range(NCORES)))
    return finalize(res.results)



# revision 13
# speedup vs baseline: 1.0928x; 1.0928x over previous
"""Trainium2 Bass kernel for nn_Loss_2482491097912 (SimCLR-style semi-supervised loss).

Strategy (8 NeuronCores, data-parallel over anchor rows):
  - Each core receives the FULL z1/z2 (pre-transposed, fp8-quantized on host)
    and its 1024-row slice of the masks, with rows ROLLED so that every
    core's local rows sit at columns [0:1024] -> one shared SPMD program.
  - On-core: projection MLP in transposed layout (HT = [feat, rows]) using
    fp8 DoubleRow matmuls (weights pre-scaled x64 to stay out of the fp8
    subnormal range), exact elu via 64*elu(u) = min(64*e^u - 64, relu(64u)),
    batched Ln/Exp norm reciprocals (no activation-table thrash), in-place
    fp8 normalization of the resident HT (x16 scale for fp8 range).
  - 4 sim matrices (S12/S21/S11/S22 rows of this core's block vs all
    columns) as fp8 DoubleRow matmuls, exp with fused row-sum accumulation
    on the Scalar engine, fused masked sums (scalar_tensor_tensor with
    accum) on the Vector engine.
  - Host combines per-core row sums / masked sums / diag dots into the two
    scalar losses (cheap numpy tail math).
"""

import math

import numpy as np
import ml_dtypes

import concourse.bass as bass
import concourse.bacc as bacc
import concourse.tile as tile
import concourse.mybir as mybir
from concourse.bass_utils import run_bass_kernel_spmd

F32 = mybir.dt.float32
BF16 = mybir.dt.bfloat16
F8 = mybir.dt.float8e4
U8 = mybir.dt.uint8
AF = mybir.ActivationFunctionType
OP = mybir.AluOpType
DR = mybir.MatmulPerfMode.DoubleRow

N = 8192
D = 512
NCORES = 8
RPC = N // NCORES          # rows per core = 1024
NBLK = RPC // 128          # row blocks per core = 8
NH = 2 * N                 # 16384 stacked rows (h1 then h2)
CCH = 2048                 # sim column chunk
NCCH = N // CCH            # 4 chunks per sim row
PCH = 512                  # projection column chunk
NPCH = NH // PCH           # 32 projection chunks

WS = 64.0                  # host weight prescale (fp8 subnormal dodge)
NS = 16.0                  # normalized-h prescale for fp8
EXP_SCALE = 2.0 / (NS * NS)  # activation scale recovering exp(2*sim)

# acc_all column layout: [4 sims x 8 blocks x 4 chunks] rowsums, then
# [4 mask-kinds x 8 blocks x 4 chunks] masked sums.
ACC_COLS = 256


def _emit(nc, tc, reps=1):
    zt = nc.dram_tensor("zt", [128, 4, NH], F8, kind="ExternalInput").ap()
    w1q = nc.dram_tensor("w1q", [128, 4, D], F8, kind="ExternalInput").ap()
    w2q = nc.dram_tensor("w2q", [128, 4, D], F8, kind="ExternalInput").ap()
    bvec = nc.dram_tensor("bvec", [128, 4, 4], F32, kind="ExternalInput").ap()
    pm = nc.dram_tensor("pm", [RPC, N], U8, kind="ExternalInput").ap()
    nm = nc.dram_tensor("nm", [RPC, N], U8, kind="ExternalInput").ap()

    acc_out = nc.dram_tensor("acc", [128, ACC_COLS], F32, kind="ExternalOutput").ap()
    d12_out = nc.dram_tensor("d12p", [1, RPC], F32, kind="ExternalOutput").ap()
    for _ in range(reps):
        _emit_body(nc, tc, zt, w1q, w2q, bvec, pm, nm, acc_out, d12_out)


def _emit_body(nc, tc, zt, w1q, w2q, bvec, pm, nm, acc_out, d12_out):
    # DRAM scratch for the [1,16384] <-> [128,128] norm layout shuffle
    nsq_dram = nc.dram_tensor([1, NH], F32, kind="Internal")
    nsq_dram = nsq_dram.ap()
    rinv_dram = nc.dram_tensor([1, NH], F32, kind="Internal")
    rinv_dram = rinv_dram.ap()
    top = tc.alloc_tile_pool(name="top", bufs=1)
    # resident normalized H^T as [128, 4 k-tiles, 16384] fp8 (16 KB/part)
    ht = top.tile([128, 4, NH], F8, name="ht")
    acc_all = top.tile([128, ACC_COLS], F32, name="acc_all")
    w1_sb = top.tile([128, 4, D], F8, name="w1_sb")
    w2_sb = top.tile([128, 4, D], F8, name="w2_sb")
    b_sb = top.tile([128, 4, 4], F32, name="b_sb")   # [b1+ln64, 64*b1, b2, ln16]
    ones_cb = top.tile([128, 1], BF16, name="ones_cb")  # bf16 lhsT for sums

    nc.sync.dma_start(w1_sb[:], w1q)
    nc.sync.dma_start(w2_sb[:], w2q)
    nc.sync.dma_start(b_sb[:], bvec)
    nc.vector.memset(ones_cb[:], 1.0)
    nc.vector.memset(acc_all[:], 0.0)

    # ---------------- Phase 1: projection (transposed layout) ----------------
    with (
        tc.tile_pool(name="pj_sb", bufs=2) as pj,
        tc.tile_pool(name="pj_norm", bufs=1) as pjn,
        tc.tile_pool(name="pp_l1", bufs=3, space="PSUM") as pp_l1,
        tc.tile_pool(name="pp_l2", bufs=2, space="PSUM") as pp_l2,
        tc.tile_pool(name="pp_n", bufs=2, space="PSUM") as pp_n,
    ):


        nsq_row = pjn.tile([1, NH], F32, name="nsq_row")

        def l1_stage(c):
            cs = c * PCH
            zt_t = pj.tile([128, 4, PCH], F8, name="zt_t", tag="zt", bufs=3)
            nc.sync.dma_start(zt_t[:], zt[:, :, cs:cs + PCH])
            gt = pj.tile([128, 4, PCH], F8, name="gt", tag="gt")
            for m in range(4):
                ms = m * 128
                l1_ps = pp_l1.tile([128, PCH], F32, name="l1_ps", tag="l1")
                for p in range(2):
                    nc.tensor.matmul(
                        l1_ps[:], w1_sb[:, 2 * p:2 * p + 2, ms:ms + 128],
                        zt_t[:, 2 * p:2 * p + 2, :],
                        start=(p == 0), stop=(p == 1), perf_mode=DR)
                # t64 = 64*exp(u); r64 = relu(64u + 64 b1); 64*elu(u)
                t64 = pj.tile([128, PCH], F32, name="t64", tag="t64")
                nc.scalar.activation(t64[:], l1_ps[:], AF.Exp,
                                     bias=b_sb[:, 0, m:m + 1], scale=1.0 / WS)
                r64 = pj.tile([128, PCH], BF16, name="r64", tag="r64")
                nc.vector.tensor_scalar(r64[:], l1_ps[:],
                                        b_sb[:, 1, m:m + 1], 0.0,
                                        op0=OP.add, op1=OP.max)
                nc.vector.scalar_tensor_tensor(
                    gt[:, m, :], t64[:], WS, r64[:],
                    op0=OP.subtract, op1=OP.min)
            return gt

        def l2_stage(c, gt):
            cs = c * PCH
            norms_ps = pp_n.tile([1, PCH], F32, name="norms_ps", tag="n")
            for m in range(4):
                ms = m * 128
                l2_ps = pp_l2.tile([128, PCH], F32, name="l2_ps", tag="l2")
                for p in range(2):
                    nc.tensor.matmul(
                        l2_ps[:], w2_sb[:, 2 * p:2 * p + 2, ms:ms + 128],
                        gt[:, 2 * p:2 * p + 2, :],
                        start=(p == 0), stop=(p == 1), perf_mode=DR)
                # hb = l2/(64*64) + b2 -> unnormalized fp8 into resident HT
                nc.vector.tensor_scalar(ht[:, m, cs:cs + PCH], l2_ps[:],
                                        1.0 / (WS * WS), b_sb[:, 2, m:m + 1],
                                        op0=OP.mult, op1=OP.add)
                sq = pj.tile([128, PCH], BF16, name="sq", tag="sq")
                nc.vector.tensor_mul(sq[:], ht[:, m, cs:cs + PCH],
                                     ht[:, m, cs:cs + PCH])
                nc.tensor.matmul(norms_ps[:], ones_cb[:], sq[:],
                                 start=(m == 0), stop=(m == 3))
            nc.vector.tensor_scalar_add(nsq_row[:, cs:cs + PCH],
                                        norms_ps[:], 0.0)

        prev = None
        for c in range(NPCH):
            gt = l1_stage(c)
            if prev is not None:
                l2_stage(prev[0], prev[1])
            prev = (c, gt)
        l2_stage(prev[0], prev[1])

        # ---- batched 16/sqrt(nsq) via Ln/Exp on a [128,128] layout ----
        nc.sync.dma_start(nsq_dram, nsq_row[:])
        nsq_sq = pjn.tile([128, 128], F32, name="nsq_sq")
        nc.sync.dma_start(nsq_sq[:], nsq_dram.rearrange("1 (p f) -> p f", p=128))
        lnv = pjn.tile([128, 128], F32, name="lnv")
        nc.scalar.activation(lnv[:], nsq_sq[:], AF.Ln)
        rsq = pjn.tile([128, 128], F32, name="rsq")
        nc.scalar.activation(rsq[:], lnv[:], AF.Exp, scale=-0.5,
                             bias=b_sb[:, 3, 0:1])
        nc.sync.dma_start(rinv_dram.rearrange("1 (p f) -> p f", p=128), rsq[:])

    # ---- normalize resident HT in place (fp8, x16) ----
    with tc.tile_pool(name="nz_sb", bufs=2) as nz:
        rinv_row = nz.tile([1, NH], F32, name="rinv_row", bufs=1)
        nc.sync.dma_start(rinv_row[:], rinv_dram)
        for c in range(NPCH):
            cs = c * PCH
            rb = nz.tile([128, PCH], F32, name="rb", tag="rb")
            nc.gpsimd.partition_broadcast(rb[:], rinv_row[:, cs:cs + PCH])
            for m in range(4):
                nc.vector.tensor_mul(ht[:, m, cs:cs + PCH],
                                     ht[:, m, cs:cs + PCH], rb[:])

    # ---------------- Phase 1.5: d12 = rowwise dot n1.n2 for local rows ------
    with (
        tc.tile_pool(name="dd_sb", bufs=2) as dd,
        tc.tile_pool(name="dd_ps", bufs=2, space="PSUM") as dd_ps,
    ):
        d12_sb = dd.tile([1, RPC], F32, name="d12_sb", bufs=1)
        for h in range(2):
            hs = h * 512
            dps = dd_ps.tile([1, 512], F32, name="dps", tag="dps")
            for k in range(4):
                mt = dd.tile([128, 512], BF16, name="mt", tag="mt")
                nc.vector.tensor_mul(
                    mt[:], ht[:, k, hs:hs + 512], ht[:, k, N + hs:N + hs + 512])
                nc.tensor.matmul(dps[:], ones_cb[:], mt[:],
                                 start=(k == 0), stop=(k == 3))
            nc.scalar.copy(d12_sb[:, hs:hs + 512], dps[:])
        nc.sync.dma_start(d12_out[:], d12_sb[:])

    # ---------------- Phase 2: sims + exp row-sums + masked sums -------------
    # sim order: (mat_idx, lhs_half, rhs_half, masked)
    sims = [
        (1, 0, 1, True),    # S12
        (2, 1, 0, True),    # S21
        (0, 0, 0, False),   # S11
        (3, 1, 1, False),   # S22
    ]
    with (
        tc.tile_pool(name="sm_sb", bufs=2) as sm,
        tc.tile_pool(name="mk_sb", bufs=2) as mk,
        tc.tile_pool(name="sm_ps", bufs=2, space="PSUM") as sm_ps,
    ):
        for b in range(NBLK):
            pm_t = mk.tile([128, N], U8, name="pm_t", tag="pm")
            nc.sync.dma_start(pm_t[:], pm[b * 128:(b + 1) * 128, :])
            nm_t = mk.tile([128, N], U8, name="nm_t", tag="nm")
            nc.sync.dma_start(nm_t[:], nm[b * 128:(b + 1) * 128, :])
            for mat, lh, rh, masked in sims:
                lc = lh * N + b * 128
                for c in range(NCCH):
                    rcs = rh * N + c * CCH
                    s_ps = sm_ps.tile([128, CCH], F32, name="s_ps", tag="s")
                    for n in range(4):
                        ns = n * 512
                        for p in range(2):
                            nc.tensor.matmul(
                                s_ps[:, ns:ns + 512],
                                ht[:, 2 * p:2 * p + 2, lc:lc + 128],
                                ht[:, 2 * p:2 * p + 2, rcs + ns:rcs + ns + 512],
                                start=(p == 0), stop=(p == 1), perf_mode=DR)
                    e_sb = sm.tile([128, CCH], BF16, name="e_sb", tag="e")
                    col = mat * 32 + b * 4 + c
                    nc.scalar.activation(
                        e_sb[:], s_ps[:], AF.Exp, scale=EXP_SCALE,
                        accum_out=acc_all[:, col:col + 1])
                    if masked:
                        mk_idx = 0 if mat == 1 else 2
                        for mki, m_t in ((mk_idx, pm_t), (mk_idx + 1, nm_t)):
                            mcol = 128 + mki * 32 + b * 4 + c
                            tsc = sm.tile([128, CCH], BF16, name="tsc",
                                          tag="tsc", bufs=1)
                            nc.vector.scalar_tensor_tensor(
                                tsc[:], e_sb[:], 1.0,
                                m_t[:, c * CCH:(c + 1) * CCH],
                                op0=OP.mult, op1=OP.mult,
                                accum_out=acc_all[:, mcol:mcol + 1])

        nc.sync.dma_start(acc_out[:], acc_all[:])
    top.release()


_CACHE = {}


def _build(reps=1):
    key = ("nc", reps)
    if key in _CACHE:
        return _CACHE[key]
    nc = bacc.Bacc("TRN2", target_bir_lowering=False, debug=False,
                   enable_asserts=False, num_devices=NCORES)
    with tile.TileContext(nc) as tc:
        _emit(nc, tc, reps=reps)
    nc.compile()
    _CACHE[key] = nc
    return nc


def prepare_in_maps(z1, z2, pos_mask, neg_mask, W1, b1, W2, b2):
    f8 = ml_dtypes.float8_e4m3

    def q8(x):
        return np.clip(x, -240.0, 240.0).astype(f8)

    def kfold(a):  # [512, M] -> [128, 4, M] k-tile layout
        return np.ascontiguousarray(
            a.reshape(4, 128, a.shape[1]).transpose(1, 0, 2))

    w1q = q8(kfold(np.ascontiguousarray(W1.T) * WS))
    w2q = q8(kfold(np.ascontiguousarray(W2.T) * WS))
    bvec = np.stack([
        (b1 + math.log(WS)).reshape(4, 128).T,
        (WS * b1).reshape(4, 128).T,
        b2.reshape(4, 128).T,
        np.full((128, 4), math.log(NS)),
    ], axis=1).astype(np.float32)          # [128, 4, 4]
    bvec = np.ascontiguousarray(bvec)
    pm_u8 = np.asarray(pos_mask).astype(np.uint8)
    nm_u8 = np.asarray(neg_mask).astype(np.uint8)

    in_maps = []
    for d in range(NCORES):
        r0 = d * RPC
        z1r = np.roll(z1, -r0, axis=0)
        z2r = np.roll(z2, -r0, axis=0)
        zt = q8(kfold(np.ascontiguousarray(
            np.concatenate([z1r, z2r], axis=0).T)))
        pm_d = np.ascontiguousarray(np.roll(pm_u8[r0:r0 + RPC], -r0, axis=1))
        nm_d = np.ascontiguousarray(np.roll(nm_u8[r0:r0 + RPC], -r0, axis=1))
        in_maps.append({"zt": zt, "w1q": w1q, "w2q": w2q, "bvec": bvec,
                        "pm": pm_d, "nm": nm_d})
    return in_maps


def finalize(results):
    """Host tail math (f64): per-core acc/d12p -> (unsup, semi)."""
    e2 = np.exp(2.0)
    rs = np.zeros((4, N))          # row sums of E11,E12,E21,E22 (global rows)
    mk = np.zeros(4)               # masked sums: p12, n12, p21, n21
    d12 = np.zeros(N)
    for d in range(NCORES):
        acc = results[d]["acc"].astype(np.float64)
        r0 = d * RPC
        for mat in range(4):
            cols = acc[:, mat * 32:(mat + 1) * 32].reshape(128, 8, 4).sum(2)
            rs[mat, r0:r0 + RPC] = cols.T.reshape(RPC)
        for mki in range(4):
            mk[mki] += acc[:, 128 + mki * 32:128 + (mki + 1) * 32].sum()
        d12[r0:r0 + RPC] = results[d]["d12p"][0].astype(np.float64)

    num = np.exp(2.0 * d12 / (NS * NS))
    rs11, rs12, rs21, rs22 = rs
    l1 = -np.log(num / (rs11 + rs12 - e2))
    l2 = -np.log(num / (rs22 + rs21 - e2))
    unsup = 0.5 * (l1 + l2).sum() / N

    tr = num.sum()
    p12, n12, p21, n21 = mk
    s1 = -np.log(p12 / (p12 + (n12 - tr)))
    s2 = -np.log(p21 / (p21 + (n21 - tr)))
    semi = 0.5 * (s1 + s2)

    return (np.float32(unsup), np.float32(semi))


def kernel(z1, z2, pos_mask, neg_mask, W1, b1, W2, b2):
    nc = _build()
    in_maps = prepare_in_maps(z1, z2, pos_mask, neg_mask, W1, b1, W2, b2)
    res = run_bass_kernel_spmd(nc, in_maps, core_ids=list(range(NCORES)))
    return finalize(res.results)
